# revision 1
# baseline (speedup 1.0000x reference)
"""Trainium2 kernel for DisplacementVectorsASU (gnn_message_passing).

Sharding: edge-shard M=4M across 8 cores (500k each, padded to 503808 =
128*96*41 slots); the small frac_coords table is replicated per core.

Gather strategy: the HW descriptor-generation engine only honors one
offset per partition for plain indirect DMA (multi-offset offset tables
lower incorrectly), so per-edge row gathers go through dma_gather
(InstDMAGatherAnt, the production MoE gather): a host-built stride-4
overlapping block table T4 (25000 x 64 f32; entry j = frac4 rows
4j..4j+15, 256B — dma_gather requires 256B-multiple elements and int16
indices, idx>>2 < 25000 fits) is gathered per tile, then a 1-of-4
on-chip select using the low 2 index bits (host-provided 0/1 mask
planes) recovers the exact node row. dma_gather writes edge i of a tile
to partition i%128, slot i//128; the host pre-permutes all per-edge
arrays into that slot order and inverse-permutes the output.

Math per edge: out_t = symmops[:, :3, :] @ [out_frac, 1]; periodic wrap
x - floor(x) built from round-to-nearest (fused +/-1.5*2^23
tensor_scalar) and an is_lt mask; result = in_frac - (wrap + trans).
"""
import sys

sys.path.insert(0, "/opt/trn_rl_repo")

import numpy as np

N_NODES = 100_000
M_TOTAL = 4_000_000
N_CORES = 8
P = 128
K = 96                  # slots per partition per tile
TILE = P * K            # 12288 edges per tile
NT = 41                 # tiles per core
M_CORE = TILE * NT      # 503808 padded edges per core
NB = 25_000             # stride-4 block-table entries
RND = 12582912.0        # 1.5 * 2^23 fp32 rounding constant

_cache = {}


def emit_tile(nc, pool, t, t4_d, iw0_d, iw1_d, mk_d, symm_d, tr_d, out_d):
    """Emit one 12288-edge tile: loads, 2 block-gathers, selects, math."""
    import concourse.mybir as mybir

    f32, i16 = mybir.dt.float32, mybir.dt.int16
    op = mybir.AluOpType

    iw0 = pool.tile((P, TILE // 16), i16, name="iw0")
    iw1 = pool.tile((P, TILE // 16), i16, name="iw1")
    mk = pool.tile((P, 4, K, 1), f32, name="mk")
    c0 = pool.tile((P, K, 64), f32, name="c0")
    c1 = pool.tile((P, K, 64), f32, name="c1")
    s = pool.tile((P, K, 16), f32, name="s")
    tr = pool.tile((P, K, 3), f32, name="tr")
    g0 = pool.tile((P, K, 4), f32, name="g0")
    g1 = pool.tile((P, K, 4), f32, name="g1")
    ta = pool.tile((P, K, 4), f32, name="ta")
    tb = pool.tile((P, K, 4), f32, name="tb")
    acc = pool.tile((P, K, 3), f32, name="acc")
    tmp = pool.tile((P, K, 3), f32, name="tmp")
    y = pool.tile((P, K, 3), f32, name="y")
    w = pool.tile((P, K, 3), f32, name="w")
    msk = pool.tile((P, K, 3), f32, name="msk")
    res = pool.tile((P, K, 3), f32, name="res")

    nc.sync.dma_start(iw0[:], iw0_d[t])
    nc.sync.dma_start(iw1[:], iw1_d[t])
    nc.sync.dma_start(mk[:].rearrange("p a k c -> p (a k c)"), mk_d[t])
    nc.sync.dma_start(s[:].rearrange("p k c -> p (k c)"), symm_d[t])
    nc.sync.dma_start(tr[:].rearrange("p k c -> p (k c)"), tr_d[t])
    nc.gpsimd.dma_gather(
        out_ap=c0[:], in_ap=t4_d[:], idxs_ap=iw0[:],
        num_idxs=TILE, num_idxs_reg=TILE, elem_size=64, single_packet=False)
    nc.gpsimd.dma_gather(
        out_ap=c1[:], in_ap=t4_d[:], idxs_ap=iw1[:],
        num_idxs=TILE, num_idxs_reg=TILE, elem_size=64, single_packet=False)

    v = nc.vector

    def select(dst, cand, mlo, mhi):
        # dst = cand[row lo], lo = mlo + 2*mhi in 0..3
        C = lambda r: cand[:, :, 4 * r:4 * r + 4]
        ML = mlo.to_broadcast((P, K, 4))
        MH = mhi.to_broadcast((P, K, 4))
        v.tensor_tensor(out=ta[:], in0=C(1), in1=C(0), op=op.subtract)
        v.tensor_tensor(out=ta[:], in0=ta[:], in1=ML, op=op.mult)
        v.tensor_tensor(out=ta[:], in0=ta[:], in1=C(0), op=op.add)
        v.tensor_tensor(out=tb[:], in0=C(3), in1=C(2), op=op.subtract)
        v.tensor_tensor(out=tb[:], in0=tb[:], in1=ML, op=op.mult)
        v.tensor_tensor(out=tb[:], in0=tb[:], in1=C(2), op=op.add)
        v.tensor_tensor(out=dst[:], in0=tb[:], in1=ta[:], op=op.subtract)
        v.tensor_tensor(out=dst[:], in0=dst[:], in1=MH, op=op.mult)
        v.tensor_tensor(out=dst[:], in0=dst[:], in1=ta[:], op=op.add)

    select(g0, c0, mk[:, 0], mk[:, 1])
    select(g1, c1, mk[:, 2], mk[:, 3])

    s4 = s[:].rearrange("p k (c j) -> p k c j", c=4)
    A = lambda j: s4[:, :, 0:3, j]
    G = lambda j: g1[:, :, j:j + 1].to_broadcast((P, K, 3))
    v.tensor_tensor(out=acc[:], in0=A(0), in1=G(0), op=op.mult)
    v.tensor_tensor(out=acc[:], in0=acc[:], in1=A(3), op=op.add)
    v.tensor_tensor(out=tmp[:], in0=A(1), in1=G(1), op=op.mult)
    v.tensor_tensor(out=acc[:], in0=acc[:], in1=tmp[:], op=op.add)
    v.tensor_tensor(out=tmp[:], in0=A(2), in1=G(2), op=op.mult)
    v.tensor_tensor(out=acc[:], in0=acc[:], in1=tmp[:], op=op.add)
    v.tensor_scalar(out=y[:], in0=acc[:], scalar1=RND, scalar2=-RND,
                    op0=op.add, op1=op.add)
    v.tensor_tensor(out=w[:], in0=acc[:], in1=y[:], op=op.subtract)
    v.tensor_scalar(out=msk[:], in0=w[:], scalar1=0.0, scalar2=None,
                    op0=op.is_lt)
    v.tensor_tensor(out=w[:], in0=w[:], in1=msk[:], op=op.add)
    v.tensor_tensor(out=w[:], in0=w[:], in1=tr[:], op=op.add)
    v.tensor_tensor(out=res[:], in0=g0[:, :, 0:3], in1=w[:], op=op.subtract)
    nc.sync.dma_start(out_d[t], res[:].rearrange("p k c -> p (k c)"))


def _build():
    if "nc" in _cache:
        return _cache["nc"]
    import concourse.mybir as mybir
    import concourse.tile as tile
    from concourse import bacc

    f32, i16 = mybir.dt.float32, mybir.dt.int16
    nc = bacc.Bacc(None, target_bir_lowering=False, debug=False)

    t4_d = nc.dram_tensor("t4", (NB, 64), f32, kind="ExternalInput")
    iw0_d = nc.dram_tensor("iw0", (NT, P, TILE // 16), i16, kind="ExternalInput")
    iw1_d = nc.dram_tensor("iw1", (NT, P, TILE // 16), i16, kind="ExternalInput")
    mk_d = nc.dram_tensor("mk", (NT, P, 4 * K), f32, kind="ExternalInput")
    symm_d = nc.dram_tensor("symm", (NT, P, K * 16), f32, kind="ExternalInput")
    tr_d = nc.dram_tensor("tr", (NT, P, K * 3), f32, kind="ExternalInput")
    out_d = nc.dram_tensor("out", (NT, P, K * 3), f32, kind="ExternalOutput")

    with tile.TileContext(nc) as tc:
        with tc.tile_pool(name="pool", bufs=2) as pool:
            for t in range(NT):
                emit_tile(nc, pool, t, t4_d, iw0_d, iw1_d, mk_d,
                          symm_d, tr_d, out_d)
    nc.compile()
    _cache["nc"] = nc
    return nc


def _prep(frac_coords, edge_indices, symmops, cell_translations):
    frac = np.asarray(frac_coords, np.float32)
    table = np.concatenate([frac, np.ones((N_NODES, 1), np.float32)], axis=1)
    # stride-4 overlapping blocks: T4[j] = frac4 rows 4j..4j+15
    flat = np.concatenate([table, np.zeros((16, 4), np.float32)]).ravel()
    T4 = np.ascontiguousarray(
        np.lib.stride_tricks.sliding_window_view(flat, 64)[::16][:NB])

    M_pad = M_CORE * N_CORES
    idx = np.zeros((2, M_pad), np.int32)
    idx[:, :M_TOTAL] = edge_indices
    symm = np.zeros((M_pad, 16), np.float32)
    symm[:M_TOTAL] = np.asarray(symmops, np.float32).reshape(M_TOTAL, 16)
    tr = np.zeros((M_pad, 3), np.float32)
    tr[:M_TOTAL] = cell_translations

    in_maps = []
    for c in range(N_CORES):
        sl = slice(c * M_CORE, (c + 1) * M_CORE)
        idc = idx[:, sl]
        # wrapped-16 int16 block indices: index i at [16g + i%16, i//16]
        i4 = (idc >> 2).astype(np.int16).reshape(2, NT, TILE // 16, 16)
        iw = np.tile(i4.transpose(0, 1, 3, 2), (1, 1, 8, 1))
        # low-bit 0/1 mask planes in slot order: edge i -> (i%128, i//128)
        lo = (idc & 3).reshape(2, NT, K, P)
        mk = np.empty((NT, P, 4, K), np.float32)
        mk[:, :, 0] = (lo[0] & 1).transpose(0, 2, 1)
        mk[:, :, 1] = (lo[0] >> 1).transpose(0, 2, 1)
        mk[:, :, 2] = (lo[1] & 1).transpose(0, 2, 1)
        mk[:, :, 3] = (lo[1] >> 1).transpose(0, 2, 1)
        # per-edge arrays into slot order
        sm = symm[sl].reshape(NT, K, P, 16).transpose(0, 2, 1, 3)
        trc = tr[sl].reshape(NT, K, P, 3).transpose(0, 2, 1, 3)
        in_maps.append({
            "t4": T4,
            "iw0": np.ascontiguousarray(iw[0]),
            "iw1": np.ascontiguousarray(iw[1]),
            "mk": np.ascontiguousarray(mk.reshape(NT, P, 4 * K)),
            "symm": np.ascontiguousarray(sm.reshape(NT, P, K * 16)),
            "tr": np.ascontiguousarray(trc.reshape(NT, P, K * 3)),
        })
    return in_maps


def kernel(frac_coords, edge_indices, symmops, cell_translations):
    from concourse.bass_utils import run_bass_kernel_spmd
    nc = _build()
    in_maps = _prep(frac_coords, edge_indices, symmops, cell_translations)
    res = run_bass_kernel_spmd(nc, in_maps, list(range(N_CORES)))
    # device output is slot order; slot (p, k) holds edge k*128 + p
    outs = []
    for c in range(N_CORES):
        o = res.results[c]["out"].reshape(NT, P, K, 3)
        outs.append(o.transpose(0, 2, 1, 3).reshape(M_CORE, 3))
    return np.concatenate(outs, axis=0)[:M_TOTAL]



# revision 2
# speedup vs baseline: 1.0359x; 1.0359x over previous
"""Trainium2 kernel for DisplacementVectorsASU — sorted-window two-pass design.

Design (see kernel2.py history): per-edge dma_gather descriptors cost ~14ns
each on real HW (~343us/tile in the old kernel), so the gather is restructured:
sort edges by node index on the host; any greedy group of <=96 consecutive
sorted edges spans <=4 frac-table rows (~99.99% at M/N=40), so the host stages
a 4-row window (12 f32) per group directly into that group's SBUF partition
and the device selects 1-of-4 per edge with copy_predicated. Host-side work is
pure layout: sort, slice, scatter (no arithmetic on float inputs).

Two passes because one sort key localizes one endpoint:
  pass 1 (sorted by dst=idx1): h = frac4[idx1]; W = wrap(S@h) + tr
  pass 2 (sorted by src=idx0): r = frac[idx0] - W
W is permuted between passes on the host.

Per-tile batching B=4 groups per partition (49152 edges/tile) to amortize
instruction overhead. Engine split (measured): GpSimd/Pool elementwise is ~10x
slower than DVE -> Pool idle; DVE does select+einsum+wrap; Act does the
unconditional select copy. One-hot mask planes are uploaded as a separate
contiguous u8 tensor (byte-strided mask reads measured ~3x slower than
contiguous), and symm is staged j-major so each einsum operand A(j) is an
inner-contiguous 3-float run.
"""
import sys

sys.path.insert(0, "/opt/trn_rl_repo")

import numpy as np

N_NODES = 100_000
M_TOTAL = 4_000_000
N_CORES = 8
P = 128
K = 96
B = 4
RND = 12582912.0

_cache = {}
_cache_nt = {}


def emit_tile1(nc, pool, t, st_d, mu_d, win_d, w_d):
    import concourse.mybir as mybir

    f32, u8 = mybir.dt.float32, mybir.dt.uint8
    op = mybir.AluOpType
    v = nc.vector

    st = pool.tile((P, B, K, 5, 3), f32, name="st")
    mu = pool.tile((P, 3, B, K, 1), u8, name="mu")
    win = pool.tile((P, B, 1, 12), f32, name="win")
    h = pool.tile((P, B, K, 3), f32, name="h")
    acc = pool.tile((P, B, K, 3), f32, name="acc")
    tmp = pool.tile((P, B, K, 3), f32, name="tmp")
    tmp2 = pool.tile((P, B, K, 3), f32, name="tmp2")
    y = pool.tile((P, B, K, 3), f32, name="y")
    w = pool.tile((P, B, K, 3), f32, name="w")
    w2 = pool.tile((P, B, K, 3), f32, name="w2")
    wv = pool.tile((P, B, K, 3), f32, name="wv")

    nc.sync.dma_start(st[:].rearrange("p b k c j -> p (b k c j)"), st_d[t])
    nc.sync.dma_start(mu[:].rearrange("p r b k a -> p (r b k a)"), mu_d[t])
    nc.sync.dma_start(win[:].rearrange("p b a c -> p (b a c)"), win_d[t])

    S = (P, B, K, 3)
    M = lambda r: mu[:, r - 1].to_broadcast(S)
    C = lambda r: win[:, :, :, 3 * r:3 * r + 3].to_broadcast(S)
    A = lambda j: st[:, :, :, j, 0:3]
    H = lambda j: h[:, :, :, j:j + 1].to_broadcast(S)
    TR = st[:, :, :, 4, 0:3]

    nc.scalar.copy(out=h[:], in_=C(0))
    v.copy_predicated(out=h[:], mask=M(1), data=C(1))
    v.copy_predicated(out=h[:], mask=M(2), data=C(2))
    v.copy_predicated(out=h[:], mask=M(3), data=C(3))

    # acc = S[:, :3] @ [h, 1]  (same accumulate order as the reference einsum)
    v.tensor_tensor(out=acc[:], in0=A(0), in1=H(0), op=op.mult)
    v.tensor_tensor(out=acc[:], in0=acc[:], in1=A(3), op=op.add)
    v.tensor_tensor(out=tmp[:], in0=A(1), in1=H(1), op=op.mult)
    v.tensor_tensor(out=acc[:], in0=acc[:], in1=tmp[:], op=op.add)
    v.tensor_tensor(out=tmp2[:], in0=A(2), in1=H(2), op=op.mult)
    v.tensor_tensor(out=acc[:], in0=acc[:], in1=tmp2[:], op=op.add)

    # W = (acc - floor(acc)) + tr
    v.tensor_scalar(out=y[:], in0=acc[:], scalar1=RND, scalar2=-RND,
                    op0=op.add, op1=op.add)
    v.tensor_tensor(out=w[:], in0=acc[:], in1=y[:], op=op.subtract)
    v.scalar_tensor_tensor(out=w2[:], in0=w[:], scalar=0.0, in1=w[:],
                           op0=op.is_lt, op1=op.add)
    v.tensor_tensor(out=wv[:], in0=w2[:], in1=TR, op=op.add)
    nc.sync.dma_start(w_d[t], wv[:].rearrange("p b k c -> p (b k c)"))


def emit_tile2(nc, pool, t, st_d, mu_d, win_d, out_d):
    import concourse.mybir as mybir

    f32, u8 = mybir.dt.float32, mybir.dt.uint8
    op = mybir.AluOpType
    v = nc.vector

    st = pool.tile((P, B, K, 3), f32, name="st2")
    mu = pool.tile((P, 3, B, K, 1), u8, name="mu2")
    win = pool.tile((P, B, 1, 12), f32, name="win2")
    h = pool.tile((P, B, K, 3), f32, name="h2")
    res = pool.tile((P, B, K, 3), f32, name="res2")

    nc.sync.dma_start(st[:].rearrange("p b k c -> p (b k c)"), st_d[t])
    nc.sync.dma_start(mu[:].rearrange("p r b k a -> p (r b k a)"), mu_d[t])
    nc.sync.dma_start(win[:].rearrange("p b a c -> p (b a c)"), win_d[t])

    S = (P, B, K, 3)
    M = lambda r: mu[:, r - 1].to_broadcast(S)
    C = lambda r: win[:, :, :, 3 * r:3 * r + 3].to_broadcast(S)
    W = st[:, :, :, 0:3]

    nc.scalar.copy(out=h[:], in_=C(0))
    v.copy_predicated(out=h[:], mask=M(1), data=C(1))
    v.copy_predicated(out=h[:], mask=M(2), data=C(2))
    v.copy_predicated(out=h[:], mask=M(3), data=C(3))

    v.tensor_tensor(out=res[:], in0=h[:], in1=W, op=op.subtract)
    nc.sync.dma_start(out_d[t], res[:].rearrange("p b k c -> p (b k c)"))


def _build1(nt):
    key = ("p1", nt)
    if key in _cache:
        return _cache[key]
    import concourse.mybir as mybir
    import concourse.tile as tile
    from concourse import bacc

    f32 = mybir.dt.float32
    nc = bacc.Bacc(None, target_bir_lowering=False, debug=False)
    u8 = mybir.dt.uint8
    st_d = nc.dram_tensor("st", (nt, P, B * K * 15), f32, kind="ExternalInput")
    mu_d = nc.dram_tensor("mu", (nt, P, 3 * B * K), u8, kind="ExternalInput")
    win_d = nc.dram_tensor("win", (nt, P, B * 12), f32, kind="ExternalInput")
    w_d = nc.dram_tensor("w", (nt, P, B * K * 3), f32, kind="ExternalOutput")
    with tile.TileContext(nc) as tc:
        with tc.tile_pool(name="pool", bufs=2) as pool:
            for t in range(nt):
                emit_tile1(nc, pool, t, st_d, mu_d, win_d, w_d)
    nc.compile()
    _cache[key] = nc
    return nc


def _build2(nt):
    key = ("p2", nt)
    if key in _cache:
        return _cache[key]
    import concourse.mybir as mybir
    import concourse.tile as tile
    from concourse import bacc

    f32 = mybir.dt.float32
    nc = bacc.Bacc(None, target_bir_lowering=False, debug=False)
    u8 = mybir.dt.uint8
    st_d = nc.dram_tensor("st", (nt, P, B * K * 3), f32, kind="ExternalInput")
    mu_d = nc.dram_tensor("mu", (nt, P, 3 * B * K), u8, kind="ExternalInput")
    win_d = nc.dram_tensor("win", (nt, P, B * 12), f32, kind="ExternalInput")
    out_d = nc.dram_tensor("out", (nt, P, B * K * 3), f32, kind="ExternalOutput")
    with tile.TileContext(nc) as tc:
        with tc.tile_pool(name="pool", bufs=2) as pool:
            for t in range(nt):
                emit_tile2(nc, pool, t, st_d, mu_d, win_d, out_d)
    nc.compile()
    _cache[key] = nc
    return nc


# --------------------------------------------------------------------------
# host-side packing
# --------------------------------------------------------------------------

def _pack_core(key_sorted):
    n = len(key_sorted)
    starts = []
    s = 0
    while s < n:
        lim = int(np.searchsorted(key_sorted, key_sorted[s] + 4, side="left"))
        starts.append(s)
        s = min(s + K, lim)
    starts = np.asarray(starts, np.int64)
    ends = np.append(starts[1:], n)
    return starts, ends - starts, key_sorted[starts]


def _stage_pass(key_all, order, frac_rows, extras):
    """Group g -> tile t = g//(P*B), partition p = (g%(P*B))//B, b = g%B.
    Flat slot = ((t*P + p)*B + b)*K + k, matching the (nt, P, B, K) layout."""
    m_core = M_TOTAL // N_CORES
    packs = []
    for c in range(N_CORES):
        eid = order[c * m_core:(c + 1) * m_core]
        key = key_all[eid]
        starts, counts, wstart = _pack_core(key)
        packs.append((eid, key, starts, counts, wstart))
    nt = max(int(np.ceil(len(p[2]) / (P * B))) for p in packs)

    E = extras.shape[1]
    stages, mus, wins, slotmaps = [], [], [], []
    for eid, key, starts, counts, wstart in packs:
        G = len(starts)
        gid = np.repeat(np.arange(G), counts)
        kpos = np.arange(len(eid)) - np.repeat(starts, counts)
        t, r = gid // (P * B), gid % (P * B)
        slot = ((t * P + r // B) * B + r % B) * K + kpos
        lidx = key - wstart[gid]
        st = np.zeros((nt * P * B * K, E), np.float32)
        st[slot] = extras[eid]
        # contiguous one-hot planes (P, 3, B, K): flat plane idx
        mu = np.zeros((nt * P, 3, B * K), np.uint8)
        pslot, bk = slot // (B * K), slot % (B * K)
        for r in (1, 2, 3):
            mu[pslot, r - 1, bk] = (lidx == r)
        t_g = np.arange(G) // (P * B)
        r_g = np.arange(G) % (P * B)
        wslot = (t_g * P + r_g // B) * B + r_g % B
        win = np.zeros((nt * P * B, 12), np.float32)
        win[wslot] = frac_rows[wstart[:, None] + np.arange(4)].reshape(G, -1)
        stages.append(st.reshape(nt, P, B * K * E))
        mus.append(mu.reshape(nt, P, 3 * B * K))
        wins.append(win.reshape(nt, P, B * 12))
        slotmaps.append((slot, eid))
    return nt, stages, mus, wins, slotmaps


def kernel(frac_coords, edge_indices, symmops, cell_translations):
    from concourse.bass_utils import run_bass_kernel_spmd

    frac = np.asarray(frac_coords, np.float32)
    idx = np.asarray(edge_indices, np.int64)
    # j-major symm slice: A(j) rows contiguous on device
    symm12 = np.ascontiguousarray(
        np.asarray(symmops, np.float32).reshape(M_TOTAL, 4, 4)[:, :3, :]
        .transpose(0, 2, 1).reshape(M_TOTAL, 12))
    tr = np.asarray(cell_translations, np.float32)
    fracpad = np.concatenate([frac, np.zeros((4, 3), np.float32)])

    order1 = np.argsort(idx[1], kind="stable")
    ex1 = np.concatenate([symm12, tr], axis=1)
    nt1, st1, mu1, win1, maps1 = _stage_pass(idx[1], order1, fracpad, ex1)
    _cache_nt["nt1"] = nt1
    nc1 = _build1(nt1)
    in_maps1 = [{"st": st1[c], "mu": mu1[c], "win": win1[c]}
                for c in range(N_CORES)]
    res1 = run_bass_kernel_spmd(nc1, in_maps1, list(range(N_CORES)))

    W_edge = np.empty((M_TOTAL, 3), np.float32)
    for c in range(N_CORES):
        slot, eid = maps1[c]
        wflat = res1.results[c]["w"].reshape(-1, 3)
        W_edge[eid] = wflat[slot]

    order0 = np.argsort(idx[0], kind="stable")
    nt2, st2, mu2, win2, maps2 = _stage_pass(idx[0], order0, fracpad, W_edge)
    _cache_nt["nt2"] = nt2
    nc2 = _build2(nt2)
    in_maps2 = [{"st": st2[c], "mu": mu2[c], "win": win2[c]}
                for c in range(N_CORES)]
    res2 = run_bass_kernel_spmd(nc2, in_maps2, list(range(N_CORES)))

    out = np.empty((M_TOTAL, 3), np.float32)
    for c in range(N_CORES):
        slot, eid = maps2[c]
        oflat = res2.results[c]["out"].reshape(-1, 3)
        out[eid] = oflat[slot]
    return out
